# revision 1
# baseline (speedup 1.0000x reference)
"""Trainium2 Bass kernel for GNN message passing (nn_MessagePassing).

Reference computation (N=5000 nodes, E=40000 edges, U=64, EF=16, 4 steps):
    edge_mats = (edge_features @ edge_kernel + edge_bias).reshape(E, 64, 64)
    h = node_features
    4x:  nb  = h[nbr]
         msg = einsum('eij,ej->ei', edge_mats, nb)
         agg = segment_sum(msg, src, N)
         h   = GRU(agg, h)        # Keras GRUCell, reset_after=True

Device algorithm (avoids materializing the 655MB edge_mats):
  Because edge_mats_e = sum_f ef[e,f] * W_f (+ bias mat), the per-step compute
  factorizes into dense matmuls around a *static* scaled-scatter matrix S:
    stage 1:  u[j, f, n] = sum_{e: src(e)=n} S_f[e] * h[nbr_e, j]
              (per 128-edge chunk: PE matmul lhsT=gathered nb [128,64],
               rhs=S chunk [128, 17*28]; PSUM-accumulated per node bin)
    stage 2:  agg^T[i, n] = sum_f W2_f^T @ u_f     (W2 = relaid edge_kernel,
              17th channel carries edge_bias with S_16 = 1)
    GRU     : feature-major matmuls + DVE/ACT elementwise, own 640 nodes only
    exchange: PE-transpose own h slice -> AllGather (steps 1..3 only)
  Edges are sharded by destination node across 8 cores; nodes are assigned to
  cores/bins by a balanced partition (host-side index relayout only, no math).
"""

import os
import sys
import time

import numpy as np

sys.path.insert(0, "/opt/trn_rl_repo")

# ---------------------------------------------------------------- constants
N, E, U, EF, STEPS, NCORES = 5000, 40000, 64, 16, 4, 8
NPC = 640                    # nodes per core (5120 = 8*640 padded)
NPAD = NPC * NCORES
WBIN = 28                    # node slots per bin
NBINS = 23                   # bins per core -> 23*28 = 644 slots
SLOTS = WBIN * NBINS
FCH = EF + 1                 # 16 edge-feature channels + 1 bias channel
SW = FCH * WBIN              # S chunk width (476 <= 512)
P = 128

_cache = {}


# ------------------------------------------------------------ preprocessing
def _preprocess(node_features, edge_features, pair_indices):
    """Pure index/relayout work (no float arithmetic on tensor values)."""
    src = np.asarray(pair_indices[:, 0], dtype=np.int64)
    nbr = np.asarray(pair_indices[:, 1], dtype=np.int64)
    ef = np.asarray(edge_features, dtype=np.float32)
    nf = np.asarray(node_features, dtype=np.float32)

    deg = np.bincount(src, minlength=N)

    # 1) assign nodes to cores, balancing edge counts (greedy by degree desc)
    order = np.argsort(-deg, kind="stable")
    core_load = np.zeros(NCORES, dtype=np.int64)
    core_cnt = np.zeros(NCORES, dtype=np.int64)
    node_core = np.empty(N, dtype=np.int64)
    for n in order:
        c = -1
        best = None
        for k in range(NCORES):
            if core_cnt[k] < NPC and (best is None or core_load[k] < best):
                best = core_load[k]
                c = k
        node_core[n] = c
        core_load[c] += deg[n]
        core_cnt[c] += 1

    # 2) per-core bin packing: NBINS bins; bins 0..NBINS-2 hold exactly WBIN
    #    nodes, the last bin holds the remainder, so real node columns are
    #    contiguous [0, n_real). Balance edges per bin greedily.
    node_col = np.empty(N, dtype=np.int64)      # column within core [0, 640)
    bins_nodes = [[[] for _ in range(NBINS)] for _ in range(NCORES)]
    for c in range(NCORES):
        nodes_c = np.where(node_core == c)[0]
        nodes_c = nodes_c[np.argsort(-deg[nodes_c], kind="stable")]
        n_real = len(nodes_c)
        # last bin's extra slots (cols >= NPC) are never exchanged -> pad only
        caps = [WBIN] * NBINS
        caps[NBINS - 1] = NPC - (NBINS - 1) * WBIN  # 24
        # exact fill targets: first fill bins to make columns contiguous we
        # only need *counts*; column = bin_id*WBIN + slot with slots filled
        # from 0.  To keep real columns 0..n_real-1 contiguous the counts
        # must be: bins 0..q-1 full (WBIN), bin q partial, rest empty?  That
        # would unbalance edges.  Instead columns are NOT required to be
        # contiguous; host unpacks by explicit (core, col) map.  So just
        # balance freely.
        bin_load = np.zeros(NBINS, dtype=np.int64)
        bin_cnt = np.zeros(NBINS, dtype=np.int64)
        for n in nodes_c:
            b = -1
            best = None
            for k in range(NBINS):
                if bin_cnt[k] < caps[k] and (best is None or bin_load[k] < best):
                    best = bin_load[k]
                    b = k
            bins_nodes[c][b].append(n)
            node_col[n] = b * WBIN + bin_cnt[b]
            bin_load[b] += deg[n]
            bin_cnt[b] += 1
        assert n_real <= NPC

    # chunks per bin (global, compile-time uniform)
    max_bin_edges = 1
    for c in range(NCORES):
        for b in range(NBINS):
            tot = int(sum(deg[n] for n in bins_nodes[c][b]))
            max_bin_edges = max(max_bin_edges, tot)
    cpb = (max_bin_edges + P - 1) // P
    nchunk = NBINS * cpb

    # 3) build S [core][128, nchunk, SW], gidx [core][128, nchunk] int32
    edges_of_node = [[] for _ in range(N)]
    for e in range(E):
        edges_of_node[src[e]].append(e)

    # global gather row of a node (in the AllGather/h_full layout)
    g_row = node_core * NPC + (node_col % WBIN) + (node_col // WBIN) * WBIN
    # note: node_col already encodes bin*WBIN + slot; global row is
    # core*NPC + node_col restricted to cols < NPC.  Columns >= NPC never
    # hold real nodes (SLOTS=644 > NPC=640 pad is sliced off at exchange),
    # so assert:
    assert node_col.max() < NPC, "real node placed in pad column"
    g_row = node_core * NPC + node_col

    S = np.zeros((NCORES, P, nchunk, SW), dtype=np.float16)
    gidx = np.zeros((NCORES, P, nchunk), dtype=np.int32)
    for c in range(NCORES):
        for b in range(NBINS):
            elist = []
            slot_of = {}
            for s_i, n in enumerate(bins_nodes[c][b]):
                slot_of[n] = s_i
                elist.extend(edges_of_node[n])
            assert len(elist) <= cpb * P
            for r, e in enumerate(elist):
                k = b * cpb + r // P
                p = r % P
                s_i = slot_of[src[e]]
                S[c, p, k, np.arange(EF) * WBIN + s_i] = ef[e]
                S[c, p, k, EF * WBIN + s_i] = 1.0
                gidx[c, p, k] = g_row[nbr[e]]

    # 4) initial h^T per core [64, SLOTS] and global nf_full [NPAD, 64]
    h0t = np.zeros((NCORES, U, SLOTS), dtype=np.float32)
    nf_full = np.zeros((NPAD, U), dtype=np.float32)
    for n in range(N):
        c = node_core[n]
        h0t[c, :, node_col[n]] = nf[n]
        nf_full[g_row[n]] = nf[n]

    # dma_gather int16 index layout: flat index i = chunk*128 + lane,
    # wrapped as idx16[i % 16, i // 16], replicated across the 8
    # 16-partition groups.
    nidx = nchunk * P
    ncol = (nidx + 15) // 16
    gidx16 = np.zeros((NCORES, P, ncol), dtype=np.int16)
    for c in range(NCORES):
        flat = gidx[c].T.reshape(-1)            # i = k*128 + p
        wrapped = np.zeros((16, ncol), dtype=np.int16)
        wrapped[np.arange(nidx) % 16, np.arange(nidx) // 16] = flat
        gidx16[c] = np.tile(wrapped, (P // 16, 1))
    return dict(S=S, gidx=gidx, gidx16=gidx16, h0t=h0t,
                h0t16=h0t.astype(np.float16),
                nf_full=nf_full, cpb=cpb,
                nchunk=nchunk, node_core=node_core, node_col=node_col)


def _prep_weights(edge_kernel, edge_bias, gru_kernel, gru_recurrent_kernel,
                  gru_bias):
    ek = np.asarray(edge_kernel, dtype=np.float32).reshape(EF, U, U)
    w2 = np.empty((U, FCH, U), dtype=np.float32)        # [j, f, i]
    w2[:, :EF, :] = np.transpose(ek, (2, 0, 1))         # w2[j,f,i]=ek[f,i,j]
    w2[:, EF, :] = np.asarray(edge_bias, dtype=np.float32).reshape(U, U).T
    w2 = w2.reshape(U, FCH * U)

    gb = np.asarray(gru_bias, dtype=np.float32)
    gbzr = np.stack([gb[0, 0:U] + gb[1, 0:U],
                     gb[0, U:2 * U] + gb[1, U:2 * U]], axis=1)   # [64, 2]
    gbh0 = gb[0, 2 * U:3 * U].reshape(U, 1)
    gbh1 = gb[1, 2 * U:3 * U].reshape(U, 1)
    return dict(w2=w2.astype(np.float16),
                gk=np.asarray(gru_kernel, dtype=np.float16),
                grk=np.asarray(gru_recurrent_kernel, dtype=np.float16),
                gbzr=gbzr, gbh0=gbh0, gbh1=gbh1)


# ------------------------------------------------------------- bass program
def _build_program(cpb, nchunk, debug=False, steps=STEPS, repeat=1, no_cc=False, no_gather=False):
    from concourse import bacc, mybir, tile
    import concourse.bass as bass
    from concourse import library_config

    f32 = mybir.dt.float32
    f16 = mybir.dt.float16
    i32 = mybir.dt.int32
    AF = mybir.ActivationFunctionType

    nc = bacc.Bacc("TRN2", target_bir_lowering=False, debug=False,
                   num_devices=NCORES)

    # ---- I/O
    t_s = nc.dram_tensor("s_mat", [P, nchunk, SW], f16, kind="ExternalInput")
    t_gidx = nc.dram_tensor("gidx16", [P, (nchunk * P + 15) // 16], mybir.dt.int16, kind="ExternalInput")
    t_h0t = nc.dram_tensor("h0t", [U, SLOTS], f32, kind="ExternalInput")
    t_h0t16 = nc.dram_tensor("h0t16", [U, SLOTS], f16, kind="ExternalInput")
    t_nf = nc.dram_tensor("nf_full", [NPAD, U], f32, kind="ExternalInput")
    t_ident = nc.dram_tensor("ident", [U, U], f32, kind="ExternalInput")
    t_w2 = nc.dram_tensor("w2", [U, FCH * U], f16, kind="ExternalInput")
    t_gk = nc.dram_tensor("gk", [U, 3 * U], f16, kind="ExternalInput")
    t_grk = nc.dram_tensor("grk", [U, 3 * U], f16, kind="ExternalInput")
    t_gbzr = nc.dram_tensor("gbzr", [U, 2], f32, kind="ExternalInput")
    t_gbh0 = nc.dram_tensor("gbh0", [U, 1], f32, kind="ExternalInput")
    t_gbh1 = nc.dram_tensor("gbh1", [U, 1], f32, kind="ExternalInput")
    t_out = nc.dram_tensor("h_out", [U, SLOTS], f32, kind="ExternalOutput")
    if debug:
        t_dbg_nb = nc.dram_tensor("dbg_nb", [P, nchunk, U], f16, kind="ExternalOutput")
        t_dbg_u = nc.dram_tensor("dbg_u", [U, FCH, SLOTS], f16, kind="ExternalOutput")
        t_dbg_agg = nc.dram_tensor("dbg_agg", [U, SLOTS], f16, kind="ExternalOutput")
        t_dbg_h1 = nc.dram_tensor("dbg_h1", [U, SLOTS], f32, kind="ExternalOutput")
        t_dbg_cc = nc.dram_tensor("dbg_cc", [NPAD, U], f16, kind="ExternalOutput")
        t_dbg_nb1 = nc.dram_tensor("dbg_nb1", [P, nchunk, U], f16, kind="ExternalOutput")

    NH = SLOTS // 2  # 322, GRU half width (>=256 keeps f32r at full rate)

    with tile.TileContext(nc) as tc:
        with (
            tc.tile_pool(name="const", bufs=1) as cpool,
            tc.tile_pool(name="work", bufs=2) as wpool,
            tc.tile_pool(name="psum", bufs=1, space="PSUM") as pp,
            tc.tile_pool(name="dram", bufs=1, space="DRAM") as dpool,
        ):
            # ---- constants into SBUF
            s_sb = cpool.tile([P, nchunk, SW], f16)
            nc.sync.dma_start(out=s_sb[:], in_=t_s[:])
            idx_sb = cpool.tile([P, (nchunk * P + 15) // 16], mybir.dt.int16)
            nc.sync.dma_start(out=idx_sb[:], in_=t_gidx[:])
            w2_sb = cpool.tile([U, FCH * U], f16)
            nc.sync.dma_start(out=w2_sb[:], in_=t_w2[:])
            gk_sb = cpool.tile([U, 3 * U], f16)
            nc.sync.dma_start(out=gk_sb[:], in_=t_gk[:])
            grk_sb = cpool.tile([U, 3 * U], f16)
            nc.sync.dma_start(out=grk_sb[:], in_=t_grk[:])
            gbzr_sb = cpool.tile([U, 2], f32)
            nc.sync.dma_start(out=gbzr_sb[:], in_=t_gbzr[:])
            gbh0_sb = cpool.tile([U, 1], f32)
            nc.sync.dma_start(out=gbh0_sb[:], in_=t_gbh0[:])
            gbh1_sb = cpool.tile([U, 1], f32)
            nc.sync.dma_start(out=gbh1_sb[:], in_=t_gbh1[:])
            ident = cpool.tile([U, U], f32)
            nc.sync.dma_start(out=ident[:], in_=t_ident[:])
            nc.gpsimd.load_library(library_config.mlp)

            hT = cpool.tile([U, SLOTS], f32, name="hT0")
            nc.sync.dma_start(out=hT[:], in_=t_h0t[:])
            hT16 = cpool.tile([U, SLOTS], f16, name="hT16_0", tag="hT16a")
            nc.sync.dma_start(out=hT16[:], in_=t_h0t16[:])

            # timing mode: `repeat` unrolled copies of the whole pipeline
            for _it in range(repeat * steps):
                _rep, step = divmod(_it, steps)
                if step == 0:
                    gather_srcs = [t_nf[:]]
                # ---------------- gather nbf[p, k, j] = h_src[idx16[k*128+p], j]
                nbf = wpool.tile([P, nchunk, U], f32, tag="nbf")
                gsrc = gather_srcs[step] if not no_cc else t_nf[:]
                if not (no_gather and _it > 0):
                  nc.gpsimd.dma_gather(
                    out_ap=nbf[:],
                    in_ap=gsrc,
                    idxs_ap=idx_sb[:],
                    num_idxs=nchunk * P,
                    num_idxs_reg=nchunk * P,
                    elem_size=U,
                    single_packet=False,
                  )
                nb = wpool.tile([P, nchunk, U], f16, tag="nb")
                nc.vector.tensor_copy(out=nb[:], in_=nbf[:])

                if debug and step == 0 and _rep == 0:
                    nc.sync.dma_start(out=t_dbg_nb[:], in_=nb[:])
                if debug and step == 1 and _rep == 0:
                    nc.sync.dma_start(out=t_dbg_nb1[:], in_=nb[:])

                # ---------------- stage 1: u[j, f, n]  (per-bin PSUM accum)
                u = wpool.tile([U, FCH, SLOTS], f16, tag="u", bufs=1)
                for b in range(NBINS):
                    ps_u = pp.tile([U, SW], f32, tag="ps_a", bufs=2)
                    for r in range(cpb):
                        k = b * cpb + r
                        nc.tensor.matmul(
                            out=ps_u[:],
                            lhsT=nb[:, k, :],
                            rhs=s_sb[:, k, :],
                            start=(r == 0), stop=(r == cpb - 1),
                        )
                    nc.scalar.copy(
                        out=u[:, :, b * WBIN:(b + 1) * WBIN],
                        in_=ps_u[:].rearrange("j (f w) -> j f w", w=WBIN),
                    )

                if debug and step == 0 and _rep == 0:
                    nc.sync.dma_start(out=t_dbg_u[:], in_=u[:])

                # ---------------- stage 2: agg^T[i, n] = sum_f W2_f^T @ u_f
                aggT = wpool.tile([U, SLOTS], f16, tag="aggT", bufs=2)
                for h in range(2):
                    ps_agg = pp.tile([U, NH], f32, tag="agg", bufs=2)
                    for f in range(FCH):
                        nc.tensor.matmul(
                            out=ps_agg[:],
                            lhsT=w2_sb[:, f * U:(f + 1) * U],
                            rhs=u[:, f, h * NH:(h + 1) * NH],
                            start=(f == 0), stop=(f == FCH - 1),
                        )
                    nc.vector.tensor_copy(
                        out=aggT[:, h * NH:(h + 1) * NH], in_=ps_agg[:])

                if debug and step == 0 and _rep == 0:
                    nc.sync.dma_start(out=t_dbg_agg[:], in_=aggT[:])

                # ---------------- GRU (feature-major, two halves)
                hT_new = cpool.tile([U, SLOTS], f32, name="hT_new",
                                    tag=f"hTn{step % 2}")
                for h in range(2):
                    sl = slice(h * NH, (h + 1) * NH)
                    ps_z = pp.tile([U, NH], f32, tag="gru_ps", bufs=4)
                    nc.tensor.matmul(out=ps_z[:],
                                     lhsT=gk_sb[:, 0:U],
                                     rhs=aggT[:, sl],
                                     start=True, stop=False)
                    nc.tensor.matmul(out=ps_z[:],
                                     lhsT=grk_sb[:, 0:U],
                                     rhs=hT16[:, sl],
                                     start=False, stop=True)
                    ps_r = pp.tile([U, NH], f32, tag="gru_ps", bufs=4)
                    nc.tensor.matmul(out=ps_r[:],
                                     lhsT=gk_sb[:, U:2 * U],
                                     rhs=aggT[:, sl],
                                     start=True, stop=False)
                    nc.tensor.matmul(out=ps_r[:],
                                     lhsT=grk_sb[:, U:2 * U],
                                     rhs=hT16[:, sl],
                                     start=False, stop=True)
                    ps_xh = pp.tile([U, NH], f32, tag="gru_ps", bufs=4)
                    nc.tensor.matmul(out=ps_xh[:],
                                     lhsT=gk_sb[:, 2 * U:3 * U],
                                     rhs=aggT[:, sl],
                                     start=True, stop=True)
                    ps_ih = pp.tile([U, NH], f32, tag="gru_ps", bufs=4)
                    nc.tensor.matmul(out=ps_ih[:],
                                     lhsT=grk_sb[:, 2 * U:3 * U],
                                     rhs=hT16[:, sl],
                                     start=True, stop=True)

                    z_sb = wpool.tile([U, NH], f32, tag="z")
                    nc.scalar.activation(out=z_sb[:], in_=ps_z[:], func=AF.Sigmoid,
                                         bias=gbzr_sb[:, 0:1])
                    r_sb = wpool.tile([U, NH], f32, tag="r")
                    nc.scalar.activation(out=r_sb[:], in_=ps_r[:], func=AF.Sigmoid,
                                         bias=gbzr_sb[:, 1:2])
                    t1 = wpool.tile([U, NH], f32, tag="t1")
                    nc.vector.tensor_scalar_add(out=t1[:], in0=ps_ih[:],
                                                scalar1=gbh1_sb[:, 0:1])
                    nc.vector.tensor_mul(out=t1[:], in0=r_sb[:], in1=t1[:])
                    nc.vector.tensor_add(out=t1[:], in0=t1[:], in1=ps_xh[:])
                    hh = wpool.tile([U, NH], f32, tag="hh")
                    nc.scalar.activation(out=hh[:], in_=t1[:], func=AF.Tanh,
                                         bias=gbh0_sb[:, 0:1])
                    d = wpool.tile([U, NH], f32, tag="d")
                    nc.vector.tensor_sub(out=d[:], in0=hT[:, sl], in1=hh[:])
                    nc.vector.tensor_mul(out=d[:], in0=z_sb[:], in1=d[:])
                    nc.vector.tensor_add(out=hT_new[:, sl], in0=hh[:], in1=d[:])

                hT16_new = cpool.tile([U, SLOTS], f16, name="hT16_new",
                                      tag=f"hT16b{step % 2}")
                nc.vector.tensor_copy(out=hT16_new[:], in_=hT_new[:])
                hT = hT_new
                hT16 = hT16_new

                # ---------------- exchange (not needed after last step)
                if step < steps - 1 and not no_cc:
                    pack = wpool.tile([P, NPC // P, U], f32, tag="pack")
                    for b in range(NPC // P):
                        ps_tr = pp.tile([P, U], f32, tag="ps_a", bufs=2)
                        nc.tensor.transpose(
                            out=ps_tr[:],
                            in_=hT[:, b * P:(b + 1) * P],
                            identity=ident[:])
                        nc.scalar.copy(out=pack[:, b, :], in_=ps_tr[:])
                    cc_in = dpool.tile([NPC, U], f32, name=f"cc_in{step}",
                                       tag=f"cc_in{step}")
                    nc.sync.dma_start(
                        out=cc_in[:].rearrange("(b p) i -> p b i", p=P),
                        in_=pack[:])
                    cc_out = dpool.tile([NPAD, U], f32, name=f"cc_out{step}",
                                        tag=f"cc_out{step}",
                                        addr_space="Shared")
                    nc.gpsimd.collective_compute(
                        "AllGather",
                        mybir.AluOpType.bypass,
                        replica_groups=[list(range(NCORES))],
                        ins=[cc_in[:].opt()],
                        outs=[cc_out[:].opt()],
                    )
                    gather_srcs.append(cc_out[:])
                    if debug and step == 0 and _rep == 0:
                        nc.sync.dma_start(out=t_dbg_h1[:], in_=hT[:])
                        nc.sync.dma_start(out=t_dbg_cc[:], in_=cc_out[:])

            nc.sync.dma_start(out=t_out[:], in_=hT[:])

    nc.compile()
    return nc


# ----------------------------------------------------------------- driver
def kernel(node_features, edge_features, pair_indices, edge_kernel, edge_bias,
           gru_kernel, gru_recurrent_kernel, gru_bias):
    prep = _preprocess(node_features, edge_features, pair_indices)
    wts = _prep_weights(edge_kernel, edge_bias, gru_kernel,
                        gru_recurrent_kernel, gru_bias)

    key = (prep["cpb"],)
    if key not in _cache:
        _cache[key] = _build_program(prep["cpb"], prep["nchunk"])
    nc = _cache[key]

    in_maps = []
    for c in range(NCORES):
        in_maps.append({
            "s_mat": prep["S"][c],
            "gidx16": prep["gidx16"][c],
            "ident": np.eye(U, dtype=np.float32),
            "h0t": prep["h0t"][c],
            "h0t16": prep["h0t16"][c],
            "nf_full": prep["nf_full"],
            "w2": wts["w2"],
            "gk": wts["gk"],
            "grk": wts["grk"],
            "gbzr": wts["gbzr"],
            "gbh0": wts["gbh0"],
            "gbh1": wts["gbh1"],
        })

    from concourse.bass_utils import run_bass_kernel_spmd
    res = run_bass_kernel_spmd(nc, in_maps, core_ids=list(range(NCORES)))
    outs = res.results

    h_final = np.empty((N, U), dtype=np.float32)
    node_core, node_col = prep["node_core"], prep["node_col"]
    for c in range(NCORES):
        ht = outs[c]["h_out"]                      # [64, SLOTS]
        sel = np.where(node_core == c)[0]
        h_final[sel] = ht[:, node_col[sel]].T
    return h_final


if __name__ == "__main__":
    sys.path.insert(0, os.path.dirname(os.path.abspath(__file__)))
    import reference

    inputs = reference.setup_inputs()
    inputs = {k: np.asarray(v) for k, v in inputs.items()}
    t0 = time.time()
    out = kernel(**inputs)
    print("kernel() wall time:", time.time() - t0)
    exp = np.asarray(reference.reference(**reference.setup_inputs()))
    err = np.abs(out - exp).max() / (np.abs(exp).max() + 1e-30)
    print("Relative error:", err)

